# revision 11
# baseline (speedup 1.0000x reference)
"""Trainium2 Bass kernel for nn_GroupConvolutionLayer2d.

Computation (see reference):
  xn = (x - mean(x, -1)) / (std(x, -1) + 1e-7)          # per-row normalize
  lm = circular_conv(lm_raw, gauss_filt(sigma=0.1))      # along last axis
  y[b, i, j] = sum_n lm[i, j, n] * xn[b, n]              # [16384, 32, 32]

Strategy: data-parallel over batch across 8 NeuronCores (2048 rows each).
The normalization is folded algebraically into the output so the matmul can
consume RAW x (host-pre-transposed, layout only):

  y[b, p] = inv_b * (sum_n x[b, n] lm[p, n]  -  mu_b * L[p])
  L[p]    = sum_n lm[p, n] = sum_m lm_raw[p, m]   (filter taps sum to 1)

which removes the per-tile on-device transpose of xn and decouples the PE
matmul stream from the stats chain entirely. Per core:
  1. warm-up junk matmuls (release HAM clock gate while DMAs stream)
  2. conv matmul (bf16): banded-circulant blocks, as before
  3. Lb[r, p] = L[p] via all-ones stationary matmul over lm_rawT tiles
  4. per 128-row tile: stats (DVE) run ahead; 16 matmuls x@lmT (PE);
     eviction fuses (z*inv - (mu*inv)*L) on DVE straight out of PSUM.
All heavy math runs on device; the host only shards/replicates inputs and
pre-transposes/casts x and lm_raw (layout + dtype only, no FLOPs).
"""

import os
import sys

import numpy as np

for _p in ("/opt/trn_rl_repo",):
    if _p not in sys.path and os.path.isdir(_p):
        sys.path.insert(0, _p)

import ml_dtypes  # noqa: E402

import concourse.bass as bass  # noqa: E402
import concourse.bass_utils as _bass_utils  # noqa: E402
import concourse.mybir as mybir  # noqa: E402
import concourse.tile as tile  # noqa: E402
from concourse import bacc  # noqa: E402
from concourse.bass_utils import run_bass_kernel_spmd  # noqa: E402

N_CORES = 8
B_FULL = 16384
BS = B_FULL // N_CORES  # 2048 rows per core
NIN = 1024
P = 1024  # 32*32 output grid, flattened
NT = BS // 128  # 16 b-tiles per core
KT = NIN // 128  # 8 contraction tiles
FILT = 33
SIGMA0 = 0.1
EPS = 1e-7

BF16 = ml_dtypes.bfloat16


def _gauss_filt() -> np.ndarray:
    t = (np.arange(FILT, dtype=np.float32) - FILT // 2) * np.float32(2.0 / FILT)
    k = np.exp(-0.5 * np.square(t / np.float32(SIGMA0)))
    return (k / k.sum()).astype(np.float32)


def _ct_blocks() -> np.ndarray:
    """Stationary blocks of C.T for the banded-circulant conv matmul.

    lm[p, n] = sum_t filt[t] * lm_raw[p, (n + t - 16) % 1024]
             = sum_m C[n, m] * lm_raw[p, m],  C[n, m] = filt[(m - n + 16) % 1024]
    With 128x128 tiling, block (mi, ni) of C.T depends only on d = (mi - ni) % 8
    and is nonzero only for d in {0, 1, 7}.
    """
    filt = _gauss_filt()
    r = np.arange(128)[:, None]
    c = np.arange(128)[None, :]
    out = np.zeros((3, 128, 128), dtype=np.float32)
    for slot, d in enumerate((0, 1, 7)):
        off = (128 * d + r - c + 16) % 1024
        out[slot] = np.where(off < FILT, filt[np.minimum(off, FILT - 1)], 0.0)
    return out


_CBT = _ct_blocks().astype(BF16)
_D_SLOT = {0: 0, 1: 1, 7: 2}


def _build_kernel_body(tc: "tile.TileContext", y_ap, xr_ap, xt_ap, lmrt_ap, cbt_ap):
    nc = tc.nc
    f32 = mybir.dt.float32
    bf16 = mybir.dt.bfloat16

    with (
        tc.tile_pool(name="const", bufs=1) as const_pool,
        tc.tile_pool(name="lm", bufs=1) as lm_pool,
        tc.tile_pool(name="xbig", bufs=1) as x_pool,
        tc.tile_pool(name="stat", bufs=12) as stat_pool,
        tc.tile_pool(name="t1p", bufs=3) as t1_pool,
        tc.tile_pool(name="yout", bufs=3) as y_pool,
        tc.tile_pool(name="psB", bufs=1, space="PSUM") as psB_pool,
        tc.tile_pool(name="pmm", bufs=3, space="PSUM") as pmm_pool,
    ):
        # ---- input DMAs, in priority order: the conv is the head of the PE
        # critical path, so lmrt is split per k-tile and ordered by when the
        # conv needs it (ni needs mi in {ni, ni+1, ni+7}); everything runs
        # behind a fixed ~7us engine preamble, so the PE self-warms on conv.
        # Issue streams run in parallel per engine: Sync carries the small,
        # urgent conv inputs; GpSimd (otherwise idle) carries the bulk x
        # loads so descriptor generation doesn't serialize the prologue.
        cbt_sb = const_pool.tile([128, 3, 128], bf16)
        nc.sync.dma_start(out=cbt_sb, in_=cbt_ap.rearrange("s r c -> r s c"))
        lmrt_sb = lm_pool.tile([128, KT, P], bf16)
        xt_sb = x_pool.tile([128, KT, BS], bf16)
        xt_re = xt_ap.rearrange("(ni r) b -> r ni b", r=128)
        xr_sb = x_pool.tile([128, NT, NIN], bf16)
        xr_re = xr_ap.rearrange("(i r) n -> r i n", r=128)

        for mi in (7, 0, 1, 2, 3, 4, 5, 6):
            nc.sync.dma_start(
                out=lmrt_sb[:, mi, :], in_=lmrt_ap[mi * 128 : (mi + 1) * 128, :]
            )
        for j in range(4):
            nc.gpsimd.dma_start(
                out=xt_sb[:, :, j * 512 : (j + 1) * 512],
                in_=xt_re[:, :, j * 512 : (j + 1) * 512],
            )
            nc.gpsimd.dma_start(
                out=xr_sb[:, j * 4 : (j + 1) * 4, :],
                in_=xr_re[:, j * 4 : (j + 1) * 4, :],
            )

        # PE warm-up on junk while the first conv inputs land: real matmuls
        # (transpose-mode doesn't tick the HAM activity monitor).
        junk = const_pool.tile([128, 512], bf16)
        nc.vector.memset(junk, 0.0)
        onesmat = const_pool.tile([128, 128], bf16)
        nc.vector.memset(onesmat, 1.0)
        warm = psB_pool.tile([128, 512], f32, tag="aux")
        for _ in range(5):
            nc.tensor.matmul(warm, lhsT=junk[:, 0:128], rhs=junk, start=True, stop=True)

        # ---- banded-circulant conv matmul -> lmT [128 n-part, ni, p] bf16,
        # with Lb[r, p] = L[p] = sum_m lm_raw[p, m] (all-ones stationary so
        # every output row carries the same sum, a free partition-broadcast)
        # interleaved per-mi to fill the DMA-paced start of the conv.
        lmT_sb = lm_pool.tile([128, KT, P], bf16)
        pLb = psB_pool.tile([128, P], f32, tag="aux")
        for ni in range(KT):
            pc = pmm_pool.tile([128, P], f32, tag="mm")
            for j, d in enumerate((0, 1, 7)):
                mi = (ni + d) % KT
                for h in range(2):
                    nc.tensor.matmul(
                        pc[:, h * 512 : (h + 1) * 512],
                        lhsT=cbt_sb[:, _D_SLOT[d], :],
                        rhs=lmrt_sb[:, mi, h * 512 : (h + 1) * 512],
                        start=(j == 0),
                        stop=(j == 2),
                    )
            nc.scalar.copy(out=lmT_sb[:, ni, :], in_=pc)
            for h in range(2):
                nc.tensor.matmul(
                    pLb[:, h * 512 : (h + 1) * 512],
                    lhsT=onesmat,
                    rhs=lmrt_sb[:, ni, h * 512 : (h + 1) * 512],
                    start=(ni == 0),
                    stop=(ni == KT - 1),
                )
        Lb_sb = const_pool.tile([128, P], f32)
        nc.scalar.copy(out=Lb_sb, in_=pLb)

        # ---- main loop over 16 batch tiles of 128 rows
        for i in range(NT):
            # stats chain (DVE/ACT) — feeds only the eviction, never the PE
            st = stat_pool.tile([128, 2, 6], f32)
            nc.vector.bn_stats(out=st[:, 0, :], in_=xr_sb[:, i, 0:512])
            nc.vector.bn_stats(out=st[:, 1, :], in_=xr_sb[:, i, 512:1024])
            mv = stat_pool.tile([128, 2], f32)
            nc.vector.bn_aggr(out=mv, in_=st)
            sd = stat_pool.tile([128, 1], f32)
            nc.scalar.activation(
                out=sd, in_=mv[:, 1:2], func=mybir.ActivationFunctionType.Sqrt
            )
            nc.vector.tensor_scalar_add(out=sd, in0=sd, scalar1=EPS)
            inv = stat_pool.tile([128, 1], f32)
            nc.vector.reciprocal(out=inv, in_=sd)
            cmu = stat_pool.tile([128, 1], f32)
            nc.vector.tensor_scalar(
                out=cmu,
                in0=mv[:, 0:1],
                scalar1=inv,
                scalar2=None,
                op0=mybir.AluOpType.mult,
            )
            t1 = t1_pool.tile([128, P], f32)
            nc.scalar.activation(
                out=t1,
                in_=Lb_sb,
                func=mybir.ActivationFunctionType.Copy,
                scale=cmu[:, 0:1],
            )

            # z_i = sum_ni xt[ni, i].T @ lmT[ni]; h inner shares the stationary
            py = pmm_pool.tile([128, P], f32, tag="mm")
            for ni in range(KT):
                for h in range(2):
                    nc.tensor.matmul(
                        py[:, h * 512 : (h + 1) * 512],
                        lhsT=xt_sb[:, ni, i * 128 : (i + 1) * 128],
                        rhs=lmT_sb[:, ni, h * 512 : (h + 1) * 512],
                        start=(ni == 0),
                        stop=(ni == KT - 1),
                    )

            # y = (z * inv) - (mu * inv) * L, fused straight out of PSUM in
            # 512-halves (each half finalizes one MM earlier). Output DMA is
            # one issue per tile on Sync; the last tile's halves are issued
            # in parallel from Vector + GpSimd to shrink the tail.
            yo = y_pool.tile([128, P], f32)
            for h in range(2):
                sl = slice(h * 512, (h + 1) * 512)
                nc.vector.scalar_tensor_tensor(
                    out=yo[:, sl],
                    in0=py[:, sl],
                    scalar=inv[:, 0:1],
                    in1=t1[:, sl],
                    op0=mybir.AluOpType.mult,
                    op1=mybir.AluOpType.subtract,
                )
            if i < NT - 1:
                nc.sync.dma_start(out=y_ap[i * 128 : (i + 1) * 128, :], in_=yo)
            else:
                nc.sync.dma_start(
                    out=y_ap[i * 128 : (i + 1) * 128, 0:512], in_=yo[:, 0:512]
                )
                nc.gpsimd.dma_start(
                    out=y_ap[i * 128 : (i + 1) * 128, 512:1024], in_=yo[:, 512:1024]
                )


_NC_CACHE = None


def _get_nc():
    global _NC_CACHE
    if _NC_CACHE is None:
        nc = bacc.Bacc(
            "TRN2", target_bir_lowering=False, debug=False, num_devices=N_CORES
        )
        xr = nc.dram_tensor("xr", [BS, NIN], mybir.dt.bfloat16, kind="ExternalInput").ap()
        xt = nc.dram_tensor("xt", [NIN, BS], mybir.dt.bfloat16, kind="ExternalInput").ap()
        lmrt = nc.dram_tensor(
            "lmrt", [NIN, P], mybir.dt.bfloat16, kind="ExternalInput"
        ).ap()
        cbt = nc.dram_tensor(
            "cbt", [3, 128, 128], mybir.dt.bfloat16, kind="ExternalInput"
        ).ap()
        y = nc.dram_tensor("y", [BS, P], mybir.dt.float32, kind="ExternalOutput").ap()
        with tile.TileContext(nc) as tc:
            _build_kernel_body(tc, y, xr, xt, lmrt, cbt)
        nc.compile()
        _NC_CACHE = nc
    return _NC_CACHE


def _in_maps(x: np.ndarray, lm_raw: np.ndarray):
    xs = np.ascontiguousarray(x, dtype=np.float32)
    xb = xs.astype(BF16)  # row-major bf16 (stats path)
    xtb = np.ascontiguousarray(xb.T)  # [n, b] bf16 (matmul stationary), layout only
    lmr = np.ascontiguousarray(lm_raw, dtype=np.float32).reshape(P, NIN)
    lmrt_b = np.ascontiguousarray(lmr.T).astype(BF16)
    return [
        {
            "xr": xb[c * BS : (c + 1) * BS],
            "xt": np.ascontiguousarray(xtb[:, c * BS : (c + 1) * BS]),
            "lmrt": lmrt_b,
            "cbt": _CBT,
        }
        for c in range(N_CORES)
    ]


def run_spmd(x: np.ndarray, lm_raw: np.ndarray, **kwargs):
    """Run the device kernel; returns (y_full, BassKernelResults)."""
    res = run_bass_kernel_spmd(
        _get_nc(), _in_maps(x, lm_raw), core_ids=list(range(N_CORES)), **kwargs
    )
    y = np.concatenate([r["y"] for r in res.results], axis=0)
    return y.reshape(B_FULL, 32, 32).astype(np.float32), res


def kernel(x: np.ndarray, lm_raw: np.ndarray) -> np.ndarray:
    y, _ = run_spmd(x, lm_raw)
    return y


# revision 14
# speedup vs baseline: 1.0719x; 1.0719x over previous
"""Trainium2 Bass kernel for nn_GroupConvolutionLayer2d.

Computation (see reference):
  xn = (x - mean(x, -1)) / (std(x, -1) + 1e-7)          # per-row normalize
  lm = circular_conv(lm_raw, gauss_filt(sigma=0.1))      # along last axis
  y[b, i, j] = sum_n lm[i, j, n] * xn[b, n]              # [16384, 32, 32]

Strategy: data-parallel over batch across 8 NeuronCores (2048 rows each).
The normalization is folded algebraically into the output so the matmul can
consume RAW x (host-pre-transposed, layout only):

  y[b, p] = inv_b * (sum_n x[b, n] lm[p, n]  -  mu_b * L[p])
  L[p]    = sum_n lm[p, n] = sum_m lm_raw[p, m]   (filter taps sum to 1)

which removes the per-tile on-device transpose of xn and decouples the PE
matmul stream from the stats chain entirely. Per core:
  1. warm-up junk matmuls (release HAM clock gate while DMAs stream)
  2. conv matmul (bf16): banded-circulant blocks, as before
  3. Lb[r, p] = L[p] via all-ones stationary matmul over lm_rawT tiles
  4. per 128-row tile: stats (DVE) run ahead; 16 matmuls x@lmT (PE);
     eviction fuses (z*inv - (mu*inv)*L) on DVE straight out of PSUM.
All heavy math runs on device; the host only shards/replicates inputs and
pre-transposes/casts x and lm_raw (layout + dtype only, no FLOPs).
"""

import os
import sys

import numpy as np

for _p in ("/opt/trn_rl_repo",):
    if _p not in sys.path and os.path.isdir(_p):
        sys.path.insert(0, _p)

import ml_dtypes  # noqa: E402

import concourse.bass as bass  # noqa: E402
import concourse.bass_utils as _bass_utils  # noqa: E402
import concourse.mybir as mybir  # noqa: E402
import concourse.tile as tile  # noqa: E402
from concourse import bacc  # noqa: E402
from concourse.bass_utils import run_bass_kernel_spmd  # noqa: E402

N_CORES = 8
B_FULL = 16384
BS = B_FULL // N_CORES  # 2048 rows per core
NIN = 1024
P = 1024  # 32*32 output grid, flattened
NT = BS // 128  # 16 b-tiles per core
KT = NIN // 128  # 8 contraction tiles
FILT = 33
SIGMA0 = 0.1
EPS = 1e-7

BF16 = ml_dtypes.bfloat16


def _gauss_filt() -> np.ndarray:
    t = (np.arange(FILT, dtype=np.float32) - FILT // 2) * np.float32(2.0 / FILT)
    k = np.exp(-0.5 * np.square(t / np.float32(SIGMA0)))
    return (k / k.sum()).astype(np.float32)


def _ct_blocks() -> np.ndarray:
    """Stationary blocks of C.T for the banded-circulant conv matmul.

    lm[p, n] = sum_t filt[t] * lm_raw[p, (n + t - 16) % 1024]
             = sum_m C[n, m] * lm_raw[p, m],  C[n, m] = filt[(m - n + 16) % 1024]
    With 128x128 tiling, block (mi, ni) of C.T depends only on d = (mi - ni) % 8
    and is nonzero only for d in {0, 1, 7}.
    """
    filt = _gauss_filt()
    r = np.arange(128)[:, None]
    c = np.arange(128)[None, :]
    out = np.zeros((3, 128, 128), dtype=np.float32)
    for slot, d in enumerate((0, 1, 7)):
        off = (128 * d + r - c + 16) % 1024
        out[slot] = np.where(off < FILT, filt[np.minimum(off, FILT - 1)], 0.0)
    return out


_CBT = _ct_blocks().astype(BF16)
_D_SLOT = {0: 0, 1: 1, 7: 2}


def _build_kernel_body(tc: "tile.TileContext", y_ap, xr_ap, xt_ap, lmrt_ap, cbt_ap):
    nc = tc.nc
    f32 = mybir.dt.float32
    bf16 = mybir.dt.bfloat16

    with (
        tc.tile_pool(name="const", bufs=1) as const_pool,
        tc.tile_pool(name="lm", bufs=1) as lm_pool,
        tc.tile_pool(name="xbig", bufs=1) as x_pool,
        tc.tile_pool(name="stat", bufs=12) as stat_pool,
        tc.tile_pool(name="t1p", bufs=3) as t1_pool,
        tc.tile_pool(name="yout", bufs=3) as y_pool,
        tc.tile_pool(name="psB", bufs=1, space="PSUM") as psB_pool,
        tc.tile_pool(name="pmm", bufs=3, space="PSUM") as pmm_pool,
    ):
        # ---- input DMAs, in priority order: the conv is the head of the PE
        # critical path, so lmrt is split per k-tile and ordered by when the
        # conv needs it (ni needs mi in {ni, ni+1, ni+7}); everything runs
        # behind a fixed ~7us engine preamble, so the PE self-warms on conv.
        # Issue streams run in parallel per engine: Sync carries the small,
        # urgent conv inputs; GpSimd (otherwise idle) carries the bulk x
        # loads so descriptor generation doesn't serialize the prologue.
        cbt_sb = const_pool.tile([128, 3, 128], bf16)
        nc.sync.dma_start(out=cbt_sb, in_=cbt_ap.rearrange("s r c -> r s c"))
        lmrt_sb = lm_pool.tile([128, KT, P], bf16)
        xt_sb = x_pool.tile([128, KT, BS], bf16)
        xt_re = xt_ap.rearrange("(ni r) b -> r ni b", r=128)
        xr_sb = x_pool.tile([128, NT, NIN], bf16)
        xr_re = xr_ap.rearrange("(i r) n -> r i n", r=128)

        # Transfers are FIFO within a queue, so queue position IS priority:
        # lmrt chunks head both queues (transferring in parallel), the bulk
        # x loads follow on GpSimd and thus cannot dilute the conv inputs.
        lmrt_re = lmrt_ap.rearrange("(mi r) p -> r mi p", r=128)

        def _lmrt_dma(eng, lo, hi):
            eng.dma_start(out=lmrt_sb[:, lo:hi, :], in_=lmrt_re[:, lo:hi, :])

        _lmrt_dma(nc.sync, 0, 2)
        _lmrt_dma(nc.sync, 7, 8)
        _lmrt_dma(nc.gpsimd, 2, 4)
        _lmrt_dma(nc.gpsimd, 4, 6)
        _lmrt_dma(nc.gpsimd, 6, 7)
        for j in range(4):
            nc.gpsimd.dma_start(
                out=xt_sb[:, :, j * 512 : (j + 1) * 512],
                in_=xt_re[:, :, j * 512 : (j + 1) * 512],
            )
            nc.gpsimd.dma_start(
                out=xr_sb[:, j * 4 : (j + 1) * 4, :],
                in_=xr_re[:, j * 4 : (j + 1) * 4, :],
            )

        # PE warm-up on junk while the first conv inputs land: real matmuls
        # (transpose-mode doesn't tick the HAM activity monitor).
        junk = const_pool.tile([128, 512], bf16)
        nc.vector.memset(junk, 0.0)
        onesmat = const_pool.tile([128, 128], bf16)
        nc.vector.memset(onesmat, 1.0)
        warm = psB_pool.tile([128, 512], f32, tag="aux")
        for _ in range(6):
            nc.tensor.matmul(warm, lhsT=junk[:, 0:128], rhs=junk, start=True, stop=True)

        # ---- banded-circulant conv matmul -> lmT [128 n-part, ni, p] bf16,
        # with Lb[r, p] = L[p] = sum_m lm_raw[p, m] (all-ones stationary so
        # every output row carries the same sum, a free partition-broadcast)
        # interleaved per-mi to fill the DMA-paced start of the conv.
        lmT_sb = lm_pool.tile([128, KT, P], bf16)
        pLb = psB_pool.tile([128, P], f32, tag="aux")
        for ni in range(KT):
            pc = pmm_pool.tile([128, P], f32, tag="mm")
            for j, d in enumerate((0, 1, 7)):
                mi = (ni + d) % KT
                for h in range(2):
                    nc.tensor.matmul(
                        pc[:, h * 512 : (h + 1) * 512],
                        lhsT=cbt_sb[:, _D_SLOT[d], :],
                        rhs=lmrt_sb[:, mi, h * 512 : (h + 1) * 512],
                        start=(j == 0),
                        stop=(j == 2),
                    )
            nc.scalar.copy(out=lmT_sb[:, ni, :], in_=pc)
            for h in range(2):
                nc.tensor.matmul(
                    pLb[:, h * 512 : (h + 1) * 512],
                    lhsT=onesmat,
                    rhs=lmrt_sb[:, ni, h * 512 : (h + 1) * 512],
                    start=(ni == 0),
                    stop=(ni == KT - 1),
                )
        Lb_sb = const_pool.tile([128, P], f32)
        nc.scalar.copy(out=Lb_sb, in_=pLb)

        # ---- main loop over 16 batch tiles of 128 rows
        for i in range(NT):
            # stats chain (DVE/ACT) — feeds only the eviction, never the PE
            st = stat_pool.tile([128, 2, 6], f32)
            nc.vector.bn_stats(out=st[:, 0, :], in_=xr_sb[:, i, 0:512])
            nc.vector.bn_stats(out=st[:, 1, :], in_=xr_sb[:, i, 512:1024])
            mv = stat_pool.tile([128, 2], f32)
            nc.vector.bn_aggr(out=mv, in_=st)
            sd = stat_pool.tile([128, 1], f32)
            nc.scalar.activation(
                out=sd, in_=mv[:, 1:2], func=mybir.ActivationFunctionType.Sqrt
            )
            nc.vector.tensor_scalar_add(out=sd, in0=sd, scalar1=EPS)
            inv = stat_pool.tile([128, 1], f32)
            nc.vector.reciprocal(out=inv, in_=sd)
            cmu = stat_pool.tile([128, 1], f32)
            nc.vector.tensor_scalar(
                out=cmu,
                in0=mv[:, 0:1],
                scalar1=inv,
                scalar2=None,
                op0=mybir.AluOpType.mult,
            )
            t1 = t1_pool.tile([128, P], f32)
            nc.scalar.activation(
                out=t1,
                in_=Lb_sb,
                func=mybir.ActivationFunctionType.Copy,
                scale=cmu[:, 0:1],
            )

            # z_i = sum_ni xt[ni, i].T @ lmT[ni]; h inner shares the stationary
            py = pmm_pool.tile([128, P], f32, tag="mm")
            for ni in range(KT):
                for h in range(2):
                    nc.tensor.matmul(
                        py[:, h * 512 : (h + 1) * 512],
                        lhsT=xt_sb[:, ni, i * 128 : (i + 1) * 128],
                        rhs=lmT_sb[:, ni, h * 512 : (h + 1) * 512],
                        start=(ni == 0),
                        stop=(ni == KT - 1),
                    )

            # y = (z * inv) - (mu * inv) * L, fused straight out of PSUM in
            # 512-halves (each half finalizes one MM earlier). Output DMA is
            # one issue per tile on Sync; the last tile's halves are issued
            # in parallel from Vector + GpSimd to shrink the tail.
            yo = y_pool.tile([128, P], f32)
            for h in range(2):
                sl = slice(h * 512, (h + 1) * 512)
                nc.vector.scalar_tensor_tensor(
                    out=yo[:, sl],
                    in0=py[:, sl],
                    scalar=inv[:, 0:1],
                    in1=t1[:, sl],
                    op0=mybir.AluOpType.mult,
                    op1=mybir.AluOpType.subtract,
                )
            if i < NT - 1:
                nc.sync.dma_start(out=y_ap[i * 128 : (i + 1) * 128, :], in_=yo)
            else:
                nc.sync.dma_start(
                    out=y_ap[i * 128 : (i + 1) * 128, 0:512], in_=yo[:, 0:512]
                )
                nc.sync.dma_start(
                    out=y_ap[i * 128 : (i + 1) * 128, 512:1024], in_=yo[:, 512:1024]
                )


_NC_CACHE = None


def _get_nc():
    global _NC_CACHE
    if _NC_CACHE is None:
        nc = bacc.Bacc(
            "TRN2", target_bir_lowering=False, debug=False, num_devices=N_CORES
        )
        xr = nc.dram_tensor("xr", [BS, NIN], mybir.dt.bfloat16, kind="ExternalInput").ap()
        xt = nc.dram_tensor("xt", [NIN, BS], mybir.dt.bfloat16, kind="ExternalInput").ap()
        lmrt = nc.dram_tensor(
            "lmrt", [NIN, P], mybir.dt.bfloat16, kind="ExternalInput"
        ).ap()
        cbt = nc.dram_tensor(
            "cbt", [3, 128, 128], mybir.dt.bfloat16, kind="ExternalInput"
        ).ap()
        y = nc.dram_tensor("y", [BS, P], mybir.dt.float32, kind="ExternalOutput").ap()
        with tile.TileContext(nc) as tc:
            _build_kernel_body(tc, y, xr, xt, lmrt, cbt)
        nc.compile()
        _NC_CACHE = nc
    return _NC_CACHE


def _in_maps(x: np.ndarray, lm_raw: np.ndarray):
    xs = np.ascontiguousarray(x, dtype=np.float32)
    xb = xs.astype(BF16)  # row-major bf16 (stats path)
    xtb = np.ascontiguousarray(xb.T)  # [n, b] bf16 (matmul stationary), layout only
    lmr = np.ascontiguousarray(lm_raw, dtype=np.float32).reshape(P, NIN)
    lmrt_b = np.ascontiguousarray(lmr.T).astype(BF16)
    return [
        {
            "xr": xb[c * BS : (c + 1) * BS],
            "xt": np.ascontiguousarray(xtb[:, c * BS : (c + 1) * BS]),
            "lmrt": lmrt_b,
            "cbt": _CBT,
        }
        for c in range(N_CORES)
    ]


def run_spmd(x: np.ndarray, lm_raw: np.ndarray, **kwargs):
    """Run the device kernel; returns (y_full, BassKernelResults)."""
    res = run_bass_kernel_spmd(
        _get_nc(), _in_maps(x, lm_raw), core_ids=list(range(N_CORES)), **kwargs
    )
    y = np.concatenate([r["y"] for r in res.results], axis=0)
    return y.reshape(B_FULL, 32, 32).astype(np.float32), res


def kernel(x: np.ndarray, lm_raw: np.ndarray) -> np.ndarray:
    y, _ = run_spmd(x, lm_raw)
    return y
